# revision 1
# baseline (speedup 1.0000x reference)
"""Multi-head attention (QKV proj + SDPA + output proj) on 8 Trainium2 cores.

Sharding: tensor-parallel over heads. 16 heads / 8 cores = 2 heads per core.
Each core computes q/k/v for its 2 heads, SDPA, and a partial output
projection against its 128-column slice of proj_w. The host sums the 8
partial projections (the all-reduce step done host-side, since this kernel
returns full outputs anyway).

Device-side layouts (per core, T = transposed so the contraction dim is on
SBUF partitions):
  xT   [1024, 4096]  x transposed (host-prepped), bf16
  wqk  [1024, 256]   [wq_c.T | wk_c.T] for the core's 2 heads, bf16
  wv   [1024, 128]   wv_c.T, bf16
  pw   [128, 1024]   proj_w[:, core cols].T, bf16
  bqk  [128, 2]      q/k biases (per-partition in qT/kT layout), f32
  out: partialT [1024, 4096] f16 = (attn_out @ proj_w_c.T).T, no biases.

The v bias and proj bias are linear post-terms: attn weights sum to 1, so
v_bias contributes qkv_b[2048:] @ proj_w.T to every row — added on host.

exp is the throughput co-bottleneck (33.5M exps/core; ACT does 1 elem/
lane/cycle @1.2GHz = 219us alone). It is split per-HEAD across engines:
  head 0: ACT true exp (activation Exp, scale folded in)
  head 1: DVE Schraudolph — u16 = round(s*A + B) where the u16 bit pattern
          IS bf16(exp(s*SCALE)):  A = 128*SCALE*log2(e),
          B = 128*(127 + 0.043).  One tensor_scalar per tile.
The split must be row-pure (all keys of a softmax row on one engine),
otherwise the Schraudolph systematic bias doesn't cancel in the softmax
ratio; per-head assignment satisfies that. Measured rms rel err ~9e-3.

Softmax skips the max-subtraction: scores have std ~1 (scale=1/8, d=64,
unit-variance q/k), so exp() stays in fp32/bf16 range with huge margin,
and u16 stays far inside [0, 65535] (|s*SCALE| <= ~13 worst case).

Normalization: softmax denominators come from a ones-column in the AV
stationary (row 64 of the av psum). Reciprocals are BATCHED: denominator
rows are DMA-gathered into four [4,512] tiles and computed in quarter-
width reciprocal instructions spread across chunk slots (DVE reciprocal
is iterative ~8cyc/elem on the free dim regardless of partition count, so
batching rows is free but wide instructions would stall the exp stream).

Scheduling: AV matmuls for exp-chunk n are emitted after chunk n+1's QK
(the PE executes in order — without this it idles ~1.7us/chunk waiting on
the exp), and all per-i postprocessing (psum drain, reciprocals,
normalize) is spread across chunk slots. Any PE idle >=3.4us re-engages
the HAM half-clock gate, which costs ~2x on everything under it.

PSUM (8 banks): sc_h0 [128,1536] (3) + sc_h1 (3) + av0 (1) + av1 (1).
"""

import numpy as np
import ml_dtypes

N_CORES = 8
SEQ = 4096
DMODEL = 1024
NHEADS = 16
DHEAD = 64
H_PER_CORE = NHEADS // N_CORES  # 2
CBLK = DMODEL // N_CORES  # 128 head-dim columns per core

IT = 512  # i (query) tile width
NI = SEQ // IT  # 8
JT = 128  # j (key) tile = psum partition dim
NJ = SEQ // JT  # 32
NCT = DMODEL // 128  # 8 contraction tiles for the projections
SCALE = DHEAD ** -0.5

# j-tiles per exp chunk: sc psum tile [128, csz*512] f32 = csz banks.
CHUNKS = [3] * 10 + [2]  # sums to NJ=32

# Schraudolph exp-as-bf16-bits constants (head 1 / DVE path)
SCHR_A = 128.0 * SCALE * 1.4426950408889634
SCHR_B = 128.0 * (127.0 + 0.043)

_CACHE = {}


def _build_nc():
    import concourse.tile as tile
    from concourse import bacc, mybir

    bf16 = mybir.dt.bfloat16
    f16 = mybir.dt.float16
    f32 = mybir.dt.float32
    u16 = mybir.dt.uint16
    Exp = mybir.ActivationFunctionType.Exp
    Mult = mybir.AluOpType.mult
    Add = mybir.AluOpType.add

    nc = bacc.Bacc(
        "TRN2",
        target_bir_lowering=False,
        debug=False,
        enable_asserts=True,
        num_devices=N_CORES,
    )

    xT = nc.dram_tensor("xT", [DMODEL, SEQ], bf16, kind="ExternalInput").ap()
    wqk = nc.dram_tensor("wqk", [DMODEL, 256], bf16, kind="ExternalInput").ap()
    wv = nc.dram_tensor("wv", [DMODEL, CBLK], bf16, kind="ExternalInput").ap()
    pw = nc.dram_tensor("pw", [CBLK, DMODEL], bf16, kind="ExternalInput").ap()
    bqk = nc.dram_tensor("bqk", [128, 2], f32, kind="ExternalInput").ap()
    partialT = nc.dram_tensor(
        "partialT", [DMODEL, SEQ], f16, kind="ExternalOutput"
    ).ap()

    with tile.TileContext(nc) as tc:
        with (
            tc.tile_pool(name="weights", bufs=1) as wpool,
            tc.tile_pool(name="xtiles", bufs=NCT) as xpool,
            tc.tile_pool(name="qk", bufs=1) as qkpool,
            tc.tile_pool(name="vaug", bufs=NJ) as vpool,
            tc.tile_pool(name="exps", bufs=2) as epool,
            tc.tile_pool(name="attn", bufs=1) as apool,
            tc.tile_pool(name="norm", bufs=4) as npool,
            tc.tile_pool(name="stage", bufs=4) as stpool,
        ):
            # ---- load weights + x (wqk_c0 + x_c0 first so the first qk
            # matmuls start as soon as possible; wv/pw are needed later) ----
            wqk_t = []
            xt = []
            for c in range(NCT):
                wqk_c = wpool.tile([128, 256], bf16, name=f"wqk_c{c}")
                nc.sync.dma_start(wqk_c[:], wqk[c * 128 : (c + 1) * 128, :])
                wqk_t.append(wqk_c)
                x_c = xpool.tile([128, SEQ], bf16, name=f"x_c{c}", tag="xc")
                # split the 8MB x load across both hwdge queues (SP + ACT)
                eng = nc.sync if c % 2 == 0 else nc.scalar
                eng.dma_start(x_c[:], xT[c * 128 : (c + 1) * 128, :])
                xt.append(x_c)
            bqk_t = wpool.tile([128, 2], f32)
            nc.sync.dma_start(bqk_t[:], bqk[:])
            # tiny dummy exp: pulls the ~2.7us ACT table load off the
            # critical path (overlaps the x DMA).
            scratch = wpool.tile([1, 1], f32)
            nc.scalar.activation(scratch[:], bqk_t[0:1, 0:1], Exp)
            wv_t = []
            for c in range(NCT):
                wv_c = wpool.tile([128, CBLK], bf16, name=f"wv_c{c}")
                nc.sync.dma_start(wv_c[:], wv[c * 128 : (c + 1) * 128, :])
                wv_t.append(wv_c)
            pw_t = wpool.tile([128, DMODEL], bf16)
            nc.sync.dma_start(pw_t[:], pw[:])

            # vaug tiles + ones-column memsets up front: DVE is idle during
            # the x DMA, and this keeps the vproj loop's per-j engine work
            # down to one copy per engine.
            vaug = []
            for j in range(NJ):
                va = vpool.tile([128, 130], bf16, name=f"vaug{j}", tag="vaug")
                nc.vector.memset(va[:, 64:65], 1.0)
                nc.vector.memset(va[:, 129:130], 1.0)
                vaug.append(va)

            # ---- QKV projections ----
            # qT/kT: [2*DHEAD=128, SEQ], stationary = w slices, moving = xT
            qT = qkpool.tile([128, SEQ], bf16)
            kT = qkpool.tile([128, SEQ], bf16)
            psqk = tc.tile_pool(name="psqk", bufs=1, space="PSUM")
            pspool = psqk.__enter__()
            for f, dest in ((0, qT), (1, kT)):
                ps = []
                for i in range(NI):
                    p = pspool.tile(
                        [128, IT], f32, name=f"qkps{f}_{i}", tag="qkps", bufs=8
                    )
                    ps.append(p)
                for c in range(NCT):
                    lhsT = wqk_t[c][:, f * 128 : (f + 1) * 128]
                    for i in range(NI):
                        nc.tensor.matmul(
                            ps[i][:],
                            lhsT,
                            xt[c][:, i * IT : (i + 1) * IT],
                            start=(c == 0),
                            stop=(c == NCT - 1),
                        )
                for i in range(NI):
                    if i % 2 == 0:
                        nc.vector.tensor_scalar_add(
                            dest[:, i * IT : (i + 1) * IT],
                            ps[i][:],
                            bqk_t[:, f : f + 1],
                        )
                    else:
                        nc.scalar.activation(
                            dest[:, i * IT : (i + 1) * IT],
                            ps[i][:],
                            mybir.ActivationFunctionType.Identity,
                            bias=bqk_t[:, f : f + 1],
                        )

            # v in natural layout [j, d] (+ ones column per head for the
            # softmax denominator): stationary = xT block, moving = wv.
            # The ones-column memsets are hoisted BEFORE the vproj loop (no
            # dependencies) so the per-j engine work is exactly one copy per
            # engine — otherwise the copies lag the matmuls and the PE idles
            # into a HAM half-clock window right at the main-loop entry.
            psqk.__exit__(None, None, None)
            psv = tc.tile_pool(name="psv", bufs=1, space="PSUM")
            pspool = psv.__enter__()
            for j in range(NJ):
                vp = pspool.tile([128, CBLK], f32, name=f"vps{j}", tag="vps", bufs=4)
                for c in range(NCT):
                    nc.tensor.matmul(
                        vp[:],
                        xt[c][:, j * JT : (j + 1) * JT],
                        wv_t[c][:],
                        start=(c == 0),
                        stop=(c == NCT - 1),
                    )
                va = vaug[j]
                if j % 2 == 0:
                    nc.vector.tensor_copy(va[:, 0:64], vp[:, 0:64])
                    nc.scalar.copy(va[:, 65:129], vp[:, 64:128])
                else:
                    nc.scalar.copy(va[:, 0:64], vp[:, 0:64])
                    nc.vector.tensor_copy(va[:, 65:129], vp[:, 64:128])

            # ---- attention ----
            # scoresT[j, i] = k . q per head. Head 0 exp on ACT, head 1 exp
            # on DVE (Schraudolph); av = v_aug.T @ expT accumulated over j;
            # row 64 of av = softmax denominator.
            psv.__exit__(None, None, None)
            psattn = tc.tile_pool(name="psattn", bufs=1, space="PSUM")
            pspool = psattn.__enter__()
            attn_outT = apool.tile([128, SEQ], bf16)
            # four 4-row den/recip tiles: engine partition offsets must be
            # 32-aligned, so each batch starts at partition 0 of its own
            # tile. Batched [4,512] reciprocals replace 16 slow 1-partition
            # ones (DVE reciprocal is iterative ~8cyc/elem on the free dim
            # regardless of partition count).
            den_b = [
                npool.tile([4, IT], f32, name=f"den{b}", tag="den", bufs=4)
                for b in range(4)
            ]
            rinv_b = [
                npool.tile([4, IT], f32, name=f"rinv{b}", tag="rinv", bufs=4)
                for b in range(4)
            ]
            avs_t = {}

            rb_t = {}

            def normalize_prep(i2, h):
                # stage the reciprocal row at partition 0 (DMA moves
                # across partitions; gpsimd broadcast cannot start at
                # a non-32-aligned partition), broadcast to 64 rows
                r = i2 * 2 + h
                rt = npool.tile([1, IT], f32, name=f"rt{r}", tag="rt", bufs=4)
                nc.sync.dma_start(rt[:], rinv_b[r // 4][r % 4 : r % 4 + 1, :])
                rb = npool.tile([64, IT], f32, name=f"rb{r}", tag="rb", bufs=4)
                nc.gpsimd.partition_broadcast(rb[:], rt[:])
                rb_t[(i2, h)] = rb

            def normalize_mul(i2, h):
                # multiply avs rows by the broadcast reciprocal -> attn_outT
                nc.vector.tensor_mul(
                    attn_outT[h * 64 : (h + 1) * 64, i2 * IT : (i2 + 1) * IT],
                    avs_t[(i2, h)][0:64, :],
                    rb_t[(i2, h)][:],
                )

            # Software-pipelined chunk loop: the AV matmuls for chunk n are
            # emitted AFTER chunk n+1's QK matmuls. The PE executes matmuls
            # in order, so without this the PE would idle ~1.7us per chunk
            # waiting for the exp (which only starts once QK(n) finishes).
            # With the reorder, exp(n) runs on ACT/DVE while the PE issues
            # QK(n+1); AV(n) is then ready to go with no stall.
            pending = None  # (i, av, et_pair, jbase, csz) awaiting AV emission

            def emit_av(p):
                _, av_p, et_p, jb, cs = p
                for t in range(cs):
                    j = jb + t
                    for h in range(2):
                        nc.tensor.matmul(
                            av_p[h][0:65, :],
                            vaug[j][:, h * 65 : h * 65 + 65],
                            et_p[h][:, t * IT : (t + 1) * IT],
                            start=(j == 0),
                            stop=(j == NJ - 1),
                        )

            def drain_av(i2, av_t):
                # drain av out of PSUM (frees the banks for the next
                # i-tile); row 64 is the softmax denominator -> DMA-gather
                # for the batched reciprocal
                for h in range(2):
                    avs = npool.tile(
                        [128, IT], f32, name=f"avs{h}_{i2}", tag="avs", bufs=16
                    )
                    if h == 0:
                        nc.scalar.copy(avs[0:65, :], av_t[h][0:65, :])
                    else:
                        nc.vector.tensor_copy(avs[0:65, :], av_t[h][0:65, :])
                    avs_t[(i2, h)] = avs
                    r = i2 * 2 + h
                    nc.sync.dma_start(
                        den_b[r // 4][r % 4 : r % 4 + 1, :], avs[64:65, :]
                    )

            for i in range(NI):
                av = [
                    pspool.tile([128, IT], f32, name=f"av0_{i}", tag="av0"),
                    pspool.tile([128, IT], f32, name=f"av1_{i}", tag="av1"),
                ]
                jbase = 0
                for ci, csz in enumerate(CHUNKS):
                    sc = [
                        pspool.tile(
                            [128, csz * IT], f32, name=f"sc{h}_{i}_{ci}",
                            tag=f"sc{h}", bufs=1,
                        )
                        for h in range(2)
                    ]
                    for t in range(csz):
                        j = jbase + t
                        for h in range(2):
                            nc.tensor.matmul(
                                sc[h][:, t * IT : (t + 1) * IT],
                                kT[h * 64 : (h + 1) * 64, j * JT : (j + 1) * JT],
                                qT[h * 64 : (h + 1) * 64, i * IT : (i + 1) * IT],
                                start=True,
                                stop=True,
                                tile_position=(h * 64, 0),
                            )
                    flushed_prev = None
                    if pending is not None:
                        emit_av(pending)
                        if pending[0] != i:  # just flushed i-1's last chunk
                            flushed_prev = pending[1]
                    # head 0: true exp on ACT (scale folded in)
                    e0 = epool.tile(
                        [128, 3 * IT], bf16, name=f"e0_{i}_{ci}", tag="e0", bufs=4
                    )
                    nc.scalar.activation(
                        e0[:, 0 : csz * IT], sc[0][:, 0 : csz * IT], Exp, scale=SCALE
                    )
                    # head 1: Schraudolph exp on DVE — u16 bits are bf16
                    e1 = epool.tile(
                        [128, 3 * IT], bf16, name=f"e1_{i}_{ci}", tag="e1", bufs=4
                    )
                    nc.vector.tensor_scalar(
                        e1[:, 0 : csz * IT].bitcast(u16),
                        sc[1][:, 0 : csz * IT],
                        SCHR_A,
                        SCHR_B,
                        Mult,
                        Add,
                    )
                    pending = (i, av, [e0, e1], jbase, csz)
                    jbase += csz

                    # Post-processing of tile i-1, spread across chunk slots
                    # so no single engine gets a burst that stalls the PE
                    # (>=3.4us PE idle re-engages the HAM half-clock gate).
                    # Emitted AFTER this chunk's exps so the exp pipeline
                    # stays primed.
                    if flushed_prev is not None:
                        drain_av(i - 1, flushed_prev)
                    p = i - 1
                    if p >= 1 and p % 2 == 1 and ci in (1, 2, 3, 4):
                        # reciprocal of 4 denominator rows in 4 quarter-
                        # width instructions (~1.1us each — DVE reciprocal
                        # is iterative ~8cyc/elem) so no single DVE burst
                        # delays the Schraudolph exp stream
                        k = p // 2
                        qtr = ci - 1
                        nc.vector.reciprocal(
                            rinv_b[k][:, qtr * 128 : (qtr + 1) * 128],
                            den_b[k][:, qtr * 128 : (qtr + 1) * 128],
                        )
                    if i >= 2:
                        # prep (DMA+gpsimd bcast) runs one slot ahead of
                        # the DVE mul so the mul never dep-stalls at the
                        # head of the DVE FIFO
                        if ci == 5:
                            normalize_prep(i - 2, 0)
                        elif ci == 6:
                            normalize_mul(i - 2, 0)
                        elif ci == 7:
                            normalize_prep(i - 2, 1)
                        elif ci == 8:
                            normalize_mul(i - 2, 1)

            emit_av(pending)
            drain_av(NI - 1, pending[1])

            psattn.__exit__(None, None, None)
            psproj = tc.tile_pool(name="psproj", bufs=1, space="PSUM")
            pspool = psproj.__enter__()
            # ---- output projection (partial, this core's 128 hd columns) ----
            # i-major, with the last recip/normalize work emitted BETWEEN
            # the first projection tiles: the PE starts projecting tiles
            # 0..5 (already normalized) immediately after the last AV — a
            # >=3.4us PE idle here would re-engage the HAM half-clock gate
            # — while DVE/gpsimd catch up on tiles 6,7 in parallel.
            def tail_norm(step):
                if step == 0:
                    for qtr in range(4):
                        nc.vector.reciprocal(
                            rinv_b[3][:, qtr * 128 : (qtr + 1) * 128],
                            den_b[3][:, qtr * 128 : (qtr + 1) * 128],
                        )
                elif step == 1:
                    normalize_prep(NI - 2, 0)
                    normalize_prep(NI - 2, 1)
                elif step == 2:
                    normalize_mul(NI - 2, 0)
                    normalize_mul(NI - 2, 1)
                elif step == 3:
                    normalize_prep(NI - 1, 0)
                    normalize_prep(NI - 1, 1)
                elif step == 4:
                    normalize_mul(NI - 1, 0)
                    normalize_mul(NI - 1, 1)

            for i in range(NI):
                if i < 5:
                    tail_norm(i)
                for cc in range(NCT):
                    lhsT = pw_t[:, cc * 128 : (cc + 1) * 128]
                    pp = pspool.tile(
                        [128, IT], f32, name=f"pp{cc}_{i}", tag="pp", bufs=8
                    )
                    nc.tensor.matmul(
                        pp[:],
                        lhsT,
                        attn_outT[:, i * IT : (i + 1) * IT],
                        start=True,
                        stop=True,
                    )
                    st = stpool.tile(
                        [128, IT], f16, name=f"st{cc}_{i}", tag="st", bufs=8
                    )
                    if cc % 8 in (1, 4, 6):
                        nc.scalar.copy(st[:], pp[:])
                        out_eng = nc.scalar
                    else:
                        nc.vector.tensor_copy(st[:], pp[:])
                        out_eng = nc.sync
                    out_eng.dma_start(
                        partialT[
                            cc * 128 : (cc + 1) * 128, i * IT : (i + 1) * IT
                        ],
                        st[:],
                    )
            psproj.__exit__(None, None, None)

    nc.compile()
    return nc


def _get_nc():
    if "nc" not in _CACHE:
        _CACHE["nc"] = _build_nc()
    return _CACHE["nc"]


def kernel(x, qkv_w, qkv_b, proj_w, proj_b):
    from concourse.bass_utils import run_bass_kernel_spmd

    nc = _get_nc()

    bf16 = ml_dtypes.bfloat16
    x2d = np.ascontiguousarray(x.reshape(SEQ, DMODEL).T).astype(bf16)  # [1024, 4096]

    in_maps = []
    for c in range(N_CORES):
        lo, hi = c * CBLK, (c + 1) * CBLK
        wq_c = qkv_w[lo:hi, :]  # [128, 1024]
        wk_c = qkv_w[DMODEL + lo : DMODEL + hi, :]
        wv_c = qkv_w[2 * DMODEL + lo : 2 * DMODEL + hi, :]
        in_maps.append(
            {
                "xT": x2d,
                "wqk": np.ascontiguousarray(
                    np.concatenate([wq_c.T, wk_c.T], axis=1)
                ).astype(bf16),
                "wv": np.ascontiguousarray(wv_c.T).astype(bf16),
                "pw": np.ascontiguousarray(proj_w[:, lo:hi].T).astype(bf16),
                "bqk": np.ascontiguousarray(
                    np.stack(
                        [qkv_b[lo:hi], qkv_b[DMODEL + lo : DMODEL + hi]], axis=1
                    )
                ).astype(np.float32),
            }
        )

    res = run_bass_kernel_spmd(nc, in_maps, core_ids=list(range(N_CORES)))

    acc = np.zeros((DMODEL, SEQ), dtype=np.float32)
    for c in range(N_CORES):
        acc += res.results[c]["partialT"].astype(np.float32)

    # host-side linear bias terms: proj bias + v-bias routed through proj
    bias = qkv_b[2 * DMODEL :].astype(np.float32) @ proj_w.T.astype(
        np.float32
    ) + proj_b.astype(np.float32)
    out = acc.T + bias[None, :]
    return out.reshape(1, SEQ, DMODEL).astype(np.float32)

